# revision 34
# baseline (speedup 1.0000x reference)
"""Trainium2 Bass kernel for the 10-class supervised-contrastive loss.

Problem shapes (hardcoded): preds [10, 2048, 128] f32, target [2048] int64,
log_vars [10] f32 -> scalar f32.

The only O(B^2) quantity is Z[c, r] = sum_{j != r} exp(cos(r, j) / T);
everything else (P/R cosine sums via class feature sums, counts, log-prob
assembly) is O(B*D) / O(B*C) and computed on the host in f32.

Sharding (8 cores, SPMD, identical program per core; B=2048 -> 16 strips of
128 rows):
  - slot 0: core c owns class c's full upper trapezoid: strip a computes
    tiles (a, b) for b >= a (136 tiles).
  - slot 1: classes 8 (cores 0-3) and 9 (cores 4-7) are split 4 ways, 34
    tiles each, with an IDENTICAL static program: fed-coordinate tiles
      row 0: cols 0..9 | row 1: cols 1..9 | row 8: cols 8..15 | row 9: 9..15
    Core j feeds the class's features rotated by 2j strips (np.roll by
    256*j rows).  The 4 rotated images of this 34-tile set partition the
    class's 136 unordered strip pairs exactly.

Each unit (strip a, cols [c0,c1)) splits into <=896-col regions; region n
lives in cp slot n%4 of one [128,3584] PSUM tile (bank 7 holds the mirror).
A static GROUPS schedule pairs regions so that most pairs are covered by a
single wide ACT (the pair's first member is always exactly 896 wide, making
the two slots contiguous) and intra-unit pairs by a single DVE row-sum
reduce.  Slot parity is arranged so pairs never wrap the 4-slot rotation:
the PE always has >=2 groups of ready main-matmul work queued, keeping its
p-state ramped.

Per region:
  PE   : bf16 matmuls C = G_a^T G[:, r0:r1] (chunks at physical 512-f32
         PSUM bank boundaries; matmul output is capped at 512 elements).
  ACT  : one Exp(C/T) -> bf16 sc per GROUP (raw diagonal included; it exps
         to ~e^14.3 which bf16 holds fine).
  GPSIMD: affine_select zeroes the 128-col diag window of each unit's
         first region.
  DVE  : tensor_reduce(X) row-sums sc (minus the diag window) per
         unit-run, one f32 column per run (direct part).
  PE   : zero-padded ones-block stationary matmuls column-sum the full sc
         region (diag tile included -> its masked colsum supplies the
         strip's own off-diagonal terms) into the mirror bank: row q
         (slot0) / 4+q (slot1) holds 512-col cell q of the class square.
         All writes are start=False accumulations onto a memset bank; rows
         below the target row accumulate exact zeros.  The diag chunk is
         emitted last so only IT depends on the affine_select; csums lag
         their group by 2 so the in-order PE queue always has ready work.

Z[row] = sum of its unit's rowsum cols + mirror[col of row]; no diagonal
corrections.

Host epilogue: masked mean log-prob from host P/R + analytic counts,
uncertainty-weighted sum.
"""

import ml_dtypes
import numpy as np

import concourse.bacc as bacc
import concourse.bass as bass
import concourse.mybir as mybir
import concourse.tile as tile
from concourse.bass_utils import run_bass_kernel_spmd

NUM_CLASSES = 10
B = 2048
D = 128
T = 0.07
BASE_T = 0.07
N_CORES = 8

f32 = mybir.dt.float32
bf16 = mybir.dt.bfloat16
np_bf16 = ml_dtypes.bfloat16

# Slot-1 static units: fed_row -> (col_start, col_end).
S1 = {0: (0, 1280), 1: (128, 1280), 8: (1024, 2048), 9: (1152, 2048)}

# G0 DMA chunk boundaries (finer up front; even indices ride the sync
# queue, odd the scalar queue, so the first compute group's inputs
# ([0,512)) are first on both queues).
G0_CHUNKS = [(0, 256), (256, 512), (512, 1024), (1024, 1536), (1536, 2048)]

# Static group schedule.  Each group is one [128,1024] cp/sc buffer (pool
# rotation depth 3) holding 1-2 regions (s, idx, r0, r1, first), covered by
# ONE ACT.  Invariants (checked below):
#   - total group width <= 1024,
#   - first regions start at the unit's c0 (diag in their first 128 cols),
#   - early groups only need early G0 DMA chunks (staged 512-wide start);
#     slot-1 units come after G1's DMA lands; small groups at the tail.
GROUPS = [
    [(0, 0, 0, 512, True)],
    [(0, 1, 128, 1024, True)],
    [(0, 0, 512, 1536, False)],
    [(0, 8, 1024, 2048, True)],
    [(0, 2, 256, 1280, True)],
    [(0, 0, 1536, 2048, False), (0, 12, 1536, 2048, True)],
    [(0, 3, 384, 1408, True)],
    [(0, 1, 1024, 2048, False)],
    [(0, 2, 1280, 2048, False)],
    [(0, 4, 512, 1536, True)],
    [(0, 9, 1152, 2048, True)],
    [(0, 3, 1408, 2048, False), (0, 13, 1664, 2048, True)],
    [(1, 0, 0, 1024, True)],
    [(0, 5, 640, 1664, True)],
    [(0, 4, 1536, 2048, False), (0, 14, 1792, 2048, True)],
    [(1, 8, 1024, 2048, True)],
    [(0, 6, 768, 1792, True)],
    [(0, 5, 1664, 2048, False), (0, 15, 1920, 2048, True)],
    [(1, 1, 128, 1152, True)],
    [(0, 7, 896, 1920, True)],
    [(0, 6, 1792, 2048, False), (1, 0, 1024, 1280, False)],
    [(0, 11, 1408, 2048, True)],
    [(1, 9, 1152, 2048, True)],
    [(0, 10, 1280, 2048, True)],
    [(0, 7, 1920, 2048, False), (1, 1, 1152, 1280, False)],
]


def _reduce_plan():
    """One DVE reduce per consecutive same-unit run within a group.
    Returns list of (group_i, sc_lo, sc_hi, rs_col->unit) entries and the
    region serial bookkeeping."""
    plan = []  # (gi, lo, hi) sc offsets within the group's sc window
    col_unit = []  # rs col -> (s, idx)
    for gi, g in enumerate(GROUPS):
        off = 0
        runs = []  # (s, idx, lo, hi, first_of_run)
        for s, idx, r0, r1, first in g:
            W = r1 - r0
            if runs and runs[-1][0] == s and runs[-1][1] == idx:
                runs[-1] = (s, idx, runs[-1][2], off + W, runs[-1][4])
            else:
                runs.append((s, idx, off, off + W, first))
            off += W
        for s, idx, lo, hi, first in runs:
            lo2 = lo + 128 if first else lo
            if hi > lo2:
                plan.append((gi, lo2, hi, len(col_unit)))
                col_unit.append((s, idx))
    return plan, col_unit


REDUCE_PLAN, RS_COL_UNIT = _reduce_plan()
N_RS = len(RS_COL_UNIT)

# Groups whose row sums ride the ACT accumulator instead of DVE (single
# diag-free region covering the whole ACT range).  Empty: the ~340ns
# READ_ACCUMULATOR on the ACT queue costs more than the DVE relief.
ACC_GROUPS = set()

TRACE = False
LAST_RESULT = None


def _build_nc():
    nc = bacc.Bacc(None, target_bir_lowering=False)

    # G0 split finer up front so the first groups' data lands ASAP.
    g0_dram = [
        nc.dram_tensor(f"g0c{k}", [128, b - a], bf16, kind="ExternalInput")
        for k, (a, b) in enumerate(G0_CHUNKS)
    ]
    g1_dram = [
        nc.dram_tensor(f"g1c{k}", [128, 512], bf16, kind="ExternalInput")
        for k in range(4)
    ]
    rs_dram = nc.dram_tensor("rs", [128, N_RS], f32, kind="ExternalOutput")
    mir_dram = nc.dram_tensor("mir", [8, 512], f32, kind="ExternalOutput")

    add = mybir.AluOpType.add
    ne = mybir.AluOpType.not_equal
    EXP = mybir.ActivationFunctionType.Exp

    with tile.TileContext(nc) as tc:
        with (
            tc.tile_pool(name="const", bufs=1) as constp,
            tc.tile_pool(name="gmat", bufs=1) as gmatp,
        ):
            # G matrices: one [128, 2048] SBUF tile per slot, filled by
            # chunked DMAs so early matmuls only wait for their own chunk.
            # G0 chunks alternate sync/scalar queues; G1 rides the gpsimd
            # queue (not needed until mid-kernel).
            G = []
            for s in range(2):
                g = gmatp.tile([128, 2048], bf16, tag=f"G{s}", name=f"G{s}")
                G.append(g)
            for k, (a, b) in enumerate(G0_CHUNKS):
                eng = nc.sync if k % 2 == 0 else nc.scalar
                eng.dma_start(G[0][:, a:b], g0_dram[k][:])
            for k in range(4):
                nc.gpsimd.dma_start(G[1][:, 512 * k : 512 * (k + 1)], g1_dram[k][:])

            # Exp-table preload: a dummy ACTIVATE on an uninitialized
            # scratch tile pulls the ~1.5us ACT table load into the DMA
            # window; emitted after the scalar-queue DMA issues.
            warm = constp.tile([128, 2], bf16, tag="warm")
            nc.scalar.activation(warm[:, 1:2], warm[:, 0:1], EXP)

            # Zero-padded ones block for the mirror column-sum matmuls:
            # opad[:, 7-r : 8] is a [128, r+1] stationary whose rows 0..r-1
            # produce exact-zero accumulands and row r the column sum.
            opad = constp.tile([128, 8], bf16, tag="opad")
            nc.vector.memset(opad[:, 0:7], 0.0)
            nc.vector.memset(opad[:, 7:8], 1.0)

            rs_sb = constp.tile([128, N_RS], f32, tag="rs")
            mir_sb = constp.tile([128, 512], f32, tag="mirsb")

            with (
                tc.tile_pool(name="scp", bufs=6) as scp,
                tc.tile_pool(name="cpp", bufs=3, space="PSUM") as cpp,
                tc.tile_pool(name="mirp", bufs=1, space="PSUM") as mirp,
            ):
                mir = mirp.tile([128, 512], f32, tag="mir", name="mir")
                # All csum matmuls accumulate with start=False, so the
                # mirror cells must begin as zeros.
                nc.vector.memset(mir[0:8, 0:512], 0.0)

                def emit_mains(cp, off, s, idx, r0, r1):
                    W = r1 - r0
                    lhsT = G[s][:, 128 * idx : 128 * idx + 128]
                    # Chunk at 512-f32 bank boundaries of the cp tile.
                    cuts = sorted({off, off + W} | {
                        b for b in (512,) if off < b < off + W
                    })
                    for lo, hi in zip(cuts, cuts[1:]):
                        nc.tensor.matmul(
                            cp[:, lo:hi],
                            lhsT,
                            G[s][:, r0 + lo - off : r0 + hi - off],
                            start=True,
                            stop=True,
                        )

                def emit_csums(sc, off, s, idx, r0, r1, first):
                    # Split [r0,r1) at the 512-cell grid; the diag chunk
                    # (first 128 cols of a first region) goes last and is
                    # the only csum depending on the affine_select.
                    cuts = sorted(
                        {r0, r1}
                        | {b for b in (512, 1024, 1536) if r0 < b < r1}
                        | ({r0 + 128} if first else set())
                    )
                    chunks = list(zip(cuts, cuts[1:]))
                    if first:
                        chunks = chunks[1:] + chunks[:1]
                    for a, b in chunks:
                        q = a // 512
                        assert b <= 512 * (q + 1)
                        row = q if s == 0 else 4 + q
                        nc.tensor.matmul(
                            mir[0 : row + 1, a - 512 * q : b - 512 * q],
                            opad[:, 7 - row : 8],
                            sc[:, off + a - r0 : off + b - r0],
                            start=False,
                            stop=True,
                            skip_group_check=True,
                        )

                lag = []  # groups awaiting csum emission (depth 2)
                red_i = 0
                for gi, g in enumerate(GROUPS):
                    Wg = sum(r[3] - r[2] for r in g)
                    assert Wg <= 1024
                    cp = cpp.tile([128, 1024], f32, tag="cp", name=f"cp{gi}")
                    off = 0
                    for r in g:
                        emit_mains(cp, off, *r[:4])
                        off += r[3] - r[2]
                    sc = scp.tile([128, 1024], bf16, tag="sc", name=f"sc{gi}")
                    # A couple of wide diag-free groups row-sum on the ACT
                    # accumulator instead of DVE (engine balance).
                    acc = None
                    if gi in ACC_GROUPS:
                        assert len(g) == 1 and not g[0][4]
                        acc = rs_sb[:, REDUCE_PLAN[red_i][3] : REDUCE_PLAN[red_i][3] + 1]
                    nc.scalar.activation(
                        sc[:, 0:Wg], cp[:, 0:Wg], EXP, scale=1.0 / T,
                        accum_out=acc,
                    )
                    off = 0
                    for s, idx, r0, r1, first in g:
                        if first:
                            nc.gpsimd.affine_select(
                                sc[:, off : off + 128],
                                sc[:, off : off + 128],
                                pattern=[[-1, 128]], compare_op=ne, fill=0.0,
                                base=0, channel_multiplier=1,
                            )
                        off += r1 - r0
                    while red_i < len(REDUCE_PLAN) and REDUCE_PLAN[red_i][0] == gi:
                        _, lo, hi, col = REDUCE_PLAN[red_i]
                        if gi not in ACC_GROUPS:
                            nc.vector.tensor_reduce(
                                rs_sb[:, col : col + 1], sc[:, lo:hi],
                                axis=mybir.AxisListType.X, op=add,
                            )
                        red_i += 1
                    lag.append((sc, g))
                    if len(lag) > 2:
                        msc, mg = lag.pop(0)
                        off = 0
                        for r in mg:
                            emit_csums(msc, off, *r)
                            off += r[3] - r[2]
                    if gi == len(GROUPS) - 4:
                        # Early rowsums are final; overlap their DMA with
                        # the tail (sync queue is idle here).
                        k = N_RS - 6
                        nc.sync.dma_start(rs_dram[:, 0:k], rs_sb[:, 0:k])
                for msc, mg in lag:
                    off = 0
                    for r in mg:
                        emit_csums(msc, off, *r)
                        off += r[3] - r[2]

                # Mirror bank -> SBUF (DMA cannot touch PSUM).
                nc.vector.tensor_copy(mir_sb[0:8, 0:512], mir[0:8, 0:512])

            nc.sync.dma_start(mir_dram[:, :], mir_sb[0:8, 0:512])
            nc.scalar.dma_start(
                rs_dram[:, N_RS - 6 : N_RS], rs_sb[:, N_RS - 6 : N_RS]
            )
    nc.finalize()
    return nc


_NC_CACHE = None


def _get_nc():
    global _NC_CACHE
    if _NC_CACHE is None:
        _NC_CACHE = _build_nc()
    return _NC_CACHE


def kernel(preds, target, log_vars):
    global LAST_RESULT
    preds = np.asarray(preds, dtype=np.float32)
    target = np.asarray(target)
    log_vars = np.asarray(log_vars, dtype=np.float32)

    onehot = (target[None, :] == np.arange(NUM_CLASSES, dtype=target.dtype)[:, None])
    onehot = onehot.astype(np.float32)  # [10, B]
    npos = onehot.sum(axis=1).astype(np.float64)  # [10]

    # Host prep: row-normalize (f32 stats), cast bf16, d-major layout.
    norms = np.sqrt((preds.astype(np.float32) ** 2).sum(axis=2, dtype=np.float32))
    ghat32 = preds / norms[:, :, None]  # [10, B, D] f32
    ghat = ghat32.astype(np_bf16)

    # Host P/R: per-row cosine sums against positives / all rows (f32).
    u_all = ghat32.sum(axis=1)  # [10, D]
    u_pos = np.einsum("cbd,cb->cd", ghat32, onehot)  # [10, D]
    P = np.einsum("cbd,cd->cb", ghat32, u_pos)  # [10, B]
    R = np.einsum("cbd,cd->cb", ghat32, u_all)  # [10, B]

    in_maps = []
    for c in range(N_CORES):
        cls1 = 8 + c // 4
        off = 256 * (c % 4)  # rotation: fed strip f = actual strip f + 2j
        im = {}
        gt0 = np.ascontiguousarray(ghat[c].T)  # [128, 2048] [d, b]
        for k, (a, b) in enumerate(G0_CHUNKS):
            im[f"g0c{k}"] = np.ascontiguousarray(gt0[:, a:b])
        gh = np.roll(ghat[cls1], -off, axis=0) if off else ghat[cls1]
        gt1 = np.ascontiguousarray(gh.T)
        for k in range(4):
            im[f"g1c{k}"] = np.ascontiguousarray(gt1[:, 512 * k : 512 * (k + 1)])
        in_maps.append(im)

    nc = _get_nc()
    res = run_bass_kernel_spmd(nc, in_maps, list(range(N_CORES)), trace=TRACE)
    LAST_RESULT = res

    # rs cols per unit.
    unit_cols = {}
    for col, u in enumerate(RS_COL_UNIT):
        unit_cols.setdefault(u, []).append(col)

    # Assemble Z (sum over j != i of exp(cos_ij / T)) from partials.
    Z = np.zeros((NUM_CLASSES, B), dtype=np.float64)
    for c in range(N_CORES):
        rs = np.asarray(res.results[c]["rs"], dtype=np.float64)
        mir = np.asarray(res.results[c]["mir"], dtype=np.float64)  # [8, 512]
        for b in range(16):
            g0 = 128 * b
            z = mir[g0 // 512, g0 % 512 : g0 % 512 + 128].copy()
            for col in unit_cols.get((0, b), []):
                z += rs[:, col]
            Z[c, g0 : g0 + 128] = z
    for cls in (8, 9):
        cores = range(0, 4) if cls == 8 else range(4, 8)
        for t in range(16):
            acc = np.zeros(128, dtype=np.float64)
            for c in cores:
                j = c % 4
                f = (t - 2 * j) % 16
                g0 = 128 * f
                mir = np.asarray(res.results[c]["mir"], dtype=np.float64)
                acc += mir[4 + g0 // 512, g0 % 512 : g0 % 512 + 128]
                if f in S1:
                    rs = np.asarray(res.results[c]["rs"], dtype=np.float64)
                    for col in unit_cols.get((1, f), []):
                        acc += rs[:, col]
            Z[cls, 128 * t : 128 * t + 128] = acc

    lab = onehot.astype(np.float64)
    masked_cos = lab * P.astype(np.float64) + (1.0 - lab) * (R - P).astype(np.float64)
    masked_logits_sum = (masked_cos - 1.0) / T
    cnt = lab * npos[:, None] + (1.0 - lab) * (B - npos[:, None]) - 1.0
    mlpp = masked_logits_sum / cnt - np.log(Z)
    losses = -(T / BASE_T) * mlpp.mean(axis=1)  # [10]
    lv = log_vars.astype(np.float64)
    final = np.sum(np.exp(-lv) * losses + lv)
    return np.float32(final)
